# revision 2
# baseline (speedup 1.0000x reference)
"""4-engine balanced attention, v2: fused mask+exp on DVE.

Engine budget per (hp, qt) block (32 score tiles of [128, 512]):
- Act: native exp on head0's 16 tiles (scale fused), plus the PV epilogue
  divisions (activation Copy with per-partition scale = 1/rowsum).
- DVE: scalar_tensor_tensor on head1's 16 tiles: int16 bits =
  round(score*23.083 + T[k,q]) where T = 16256 (unmasked: Schraudolph
  bias; constant bias shift cancels in softmax) or -29952 (masked: bits
  land in [-31060, -28844] -> bf16 negative denormals ~ -1e-29 ~ 0).
  One op applies exp AND mask. Plus the rowsum reciprocals.
- Pool: bf16 mask-mults for head0 j < POOL0 (gpsimd tensor_tensor).
- PE: scores (64-contraction quadrants), PV (P-stationary, ones column),
  and identity-matmul additive mask (-10000) for head0 j >= 16-ID0.
"""

import numpy as np
import ml_dtypes

B, H, S, DK = 4, 16, 2048, 64
NCORES = 8
HPC = H * B // NCORES   # 8 heads per core
NPAIR = HPC // 2        # 4 head pairs
QT = 512
NQT = S // QT
KT = 128
NKT = S // KT           # 16
VE = DK + 1
SCALE = 1.0 / float(np.sqrt(DK))
LOG2E = 1.4426950408889634
SCH_A = SCALE * LOG2E * 128.0        # 23.0831

_BF16 = ml_dtypes.bfloat16

_CACHE = {}

# head0 mask split: j < POOL0 multiplied on Pool, j in [POOL0, 16-ID0) on
# DVE, j >= 16-ID0 additively masked by PE identity matmuls.
POOL0 = 9
ID0 = 7

# score j-groups: head0 uses the 4-bank ring (Act), head1 the 3-bank ring (DVE)
G0 = [(0, 4), (4, 4), (8, 4), (12, 4)]
G1 = [(0, 3), (3, 3), (6, 3), (9, 3), (12, 3), (15, 1)]


def _build_nc(reps=1):
    import concourse.mybir as mybir
    import concourse.tile as tile
    from concourse import bacc
    from contextlib import ExitStack

    dt = mybir.dt
    nc = bacc.Bacc()
    AF = mybir.ActivationFunctionType
    Alu = mybir.AluOpType

    q2 = nc.declare_dram_parameter("q2", [NPAIR, 2 * DK, S], dt.bfloat16, isOutput=False)
    k2 = nc.declare_dram_parameter("k2", [NPAIR, 2 * DK, S], dt.bfloat16, isOutput=False)
    vex = nc.declare_dram_parameter("vex", [S, HPC, VE], dt.bfloat16, isOutput=False)
    # 0/1 mask rows for head0 j < POOL0
    maskT = nc.declare_dram_parameter("maskT", [POOL0 * KT, S], dt.bfloat16, isOutput=False)
    # additive -10000/0 rows for head0 j >= 16-ID0
    maddT = nc.declare_dram_parameter("maddT", [ID0 * KT, S], dt.bfloat16, isOutput=False)
    # Schraudolph bias-mask (16256 / -29952) for head1, all rows
    exT = nc.declare_dram_parameter("exT", [S, S], dt.bfloat16, isOutput=False)
    ident = nc.declare_dram_parameter("ident", [KT, KT], dt.bfloat16, isOutput=False)
    out = nc.declare_dram_parameter("out", [HPC, S, DK], dt.float32, isOutput=True)

    jid0 = NKT - ID0

    with tile.TileContext(nc) as tc, ExitStack() as ctx:
        const = ctx.enter_context(tc.tile_pool(name="const", bufs=1))
        maskp = ctx.enter_context(tc.tile_pool(name="maskp", bufs=2))
        qp = ctx.enter_context(tc.tile_pool(name="qp", bufs=2))
        pp = ctx.enter_context(tc.tile_pool(name="pp", bufs=2))
        epi = ctx.enter_context(tc.tile_pool(name="epi", bufs=3))
        psA = ctx.enter_context(tc.tile_pool(name="psA", bufs=1, space="PSUM"))
        psB = ctx.enter_context(tc.tile_pool(name="psB", bufs=1, space="PSUM"))
        pvps = ctx.enter_context(tc.tile_pool(name="pvps", bufs=1, space="PSUM"))

        warm = const.tile([1, 2], dt.float32)
        nc.vector.memset(warm, 0.0)
        nc.scalar.activation(out=warm, in_=warm, func=AF.Exp)

        id_sb = const.tile([KT, KT], dt.bfloat16)
        nc.sync.dma_start(out=id_sb, in_=ident[:, :])

        # K^T head pairs packed [128, pair, s]; pair 0 first for fast start
        k_sb = const.tile([2 * DK, NPAIR, S], dt.bfloat16)
        nc.sync.dma_start(out=k_sb[:, 0, :], in_=k2[0])
        nc.sync.dma_start(
            out=k_sb[:, 1:, :], in_=k2[1:].rearrange("h d s -> d h s")
        )

        v_sb = const.tile([KT, NKT, HPC, VE], dt.bfloat16)
        nc.sync.dma_start(out=v_sb, in_=vex.rearrange("(j p) h e -> p j h e", p=KT))

        state = {"prev": None}   # ((p0, p1), hp, qt)

        def emit_pv_chain(st, chain):
            p_handles, hp_prev, qt_prev = st
            hc, half = chain // 2, chain % 2
            h = 2 * hp_prev + hc
            p_sb = p_handles[hc]
            pv = pvps.tile([KT, 2, VE], dt.float32, tag="pv")
            for qb in range(2):
                qcol = (half * 2 + qb) * KT
                for j in range(NKT):
                    nc.tensor.matmul(
                        out=pv[:, qb, :],
                        lhsT=p_sb[:, j, qcol:qcol + KT],
                        rhs=v_sb[:, j, h, :],
                        start=(j == 0),
                        stop=(j == NKT - 1),
                    )
            rec = epi.tile([KT, 2], dt.float32, tag="rec")
            nc.vector.reciprocal(rec, pv[:, :, DK])
            outf = epi.tile([KT, 2, DK], dt.float32, tag="outf")
            for qb in range(2):
                nc.vector.tensor_scalar_mul(
                    outf[:, qb, :], pv[:, qb, 0:DK], rec[:, qb:qb + 1]
                )
            q0 = qt_prev * QT + half * 2 * KT
            nc.sync.dma_start(
                out=out[h, q0:q0 + 2 * KT, :].rearrange("(qb p) d -> p qb d", p=KT),
                in_=outf,
            )

        def emit_rep():
            for qt in range(NQT):
                m_sb = maskp.tile([KT, POOL0, QT], dt.bfloat16, tag="m")
                nc.sync.dma_start(
                    out=m_sb,
                    in_=maskT[:, qt * QT:(qt + 1) * QT]
                    .rearrange("(j p) q -> p j q", p=KT),
                )
                ma_sb = maskp.tile([KT, ID0, QT], dt.bfloat16, tag="ma")
                nc.sync.dma_start(
                    out=ma_sb,
                    in_=maddT[:, qt * QT:(qt + 1) * QT]
                    .rearrange("(j p) q -> p j q", p=KT),
                )
                ex_sb = maskp.tile([KT, NKT, QT], dt.bfloat16, tag="ex")
                nc.sync.dma_start(
                    out=ex_sb,
                    in_=exT[:, qt * QT:(qt + 1) * QT]
                    .rearrange("(j p) q -> p j q", p=KT),
                )
                q_all = qp.tile([2 * DK, NPAIR, QT], dt.bfloat16)
                nc.sync.dma_start(
                    out=q_all,
                    in_=q2[:, :, qt * QT:(qt + 1) * QT].rearrange("h d q -> d h q"),
                )
                for hp in range(NPAIR):
                    p0 = pp.tile([KT, NKT, QT], dt.bfloat16, tag="p0")
                    p1 = pp.tile([KT, NKT, QT], dt.bfloat16, tag="p1")
                    p_cur = (p0, p1)

                    def emit_group(a, j0, nj, ps_pool, width):
                        sc = ps_pool.tile([KT, width], dt.float32, tag="sc")
                        for u in range(nj):
                            j = j0 + u
                            if a == 0 and j >= jid0:
                                nc.tensor.matmul(
                                    out=sc[:, u * QT:(u + 1) * QT],
                                    lhsT=id_sb,
                                    rhs=ma_sb[:, j - jid0, :],
                                    start=True,
                                    stop=False,
                                )
                            nc.tensor.matmul(
                                out=sc[:, u * QT:(u + 1) * QT],
                                lhsT=k_sb[64 * a:64 * a + 64, hp,
                                          j * KT:(j + 1) * KT],
                                rhs=q_all[64 * a:64 * a + 64, hp, :],
                                start=not (a == 0 and j >= jid0),
                                stop=True,
                            )
                        dst = p_cur[a][:, j0:j0 + nj, :].rearrange(
                            "p a q -> p (a q)")
                        if a == 1:
                            nc.vector.scalar_tensor_tensor(
                                out=dst.bitcast(dt.int16),
                                in0=sc[:, 0:nj * QT],
                                scalar=SCH_A,
                                in1=ex_sb[:, j0:j0 + nj, :].rearrange(
                                    "p a q -> p (a q)"),
                                op0=Alu.mult,
                                op1=Alu.add,
                            )
                        else:
                            nc.scalar.activation(
                                out=dst,
                                in_=sc[:, 0:nj * QT],
                                func=AF.Exp,
                                scale=SCALE,
                            )

                    def emit_mult(j0, nj, eng):
                        dst = p_cur[0][:, j0:j0 + nj, :].rearrange(
                            "p a q -> p (a q)")
                        eng.tensor_tensor(
                            out=dst, in0=dst,
                            in1=m_sb[:, j0:j0 + nj, :].rearrange(
                                "p a q -> p (a q)"),
                            op=Alu.mult,
                        )

                    prev = state["prev"]
                    emit_group(0, *G0[0], psA, 4 * QT)
                    emit_group(1, *G1[0], psB, 3 * QT)
                    emit_group(0, *G0[1], psA, 4 * QT)
                    emit_mult(0, 4, nc.gpsimd)       # head0 j0-3 on Pool
                    emit_group(1, *G1[1], psB, 3 * QT)
                    if prev is not None:
                        emit_pv_chain(prev, 0)
                    emit_group(0, *G0[2], psA, 4 * QT)
                    emit_mult(4, 4, nc.gpsimd)       # head0 j4-7 on Pool
                    emit_group(1, *G1[2], psB, 3 * QT)
                    if prev is not None:
                        emit_pv_chain(prev, 1)
                    emit_group(0, *G0[3], psA, 4 * QT)
                    emit_group(1, *G1[3], psB, 3 * QT)
                    emit_mult(8, POOL0 - 8, nc.gpsimd)   # head0 j8 on Pool
                    if prev is not None:
                        emit_pv_chain(prev, 2)
                    emit_group(1, *G1[4], psB, 3 * QT)
                    emit_group(1, *G1[5], psB, 1 * QT)
                    if jid0 > POOL0:                 # leftover on DVE
                        emit_mult(POOL0, jid0 - POOL0, nc.vector)
                    if prev is not None:
                        emit_pv_chain(prev, 3)

                    state["prev"] = (p_cur, hp, qt)

        for _r in range(reps):
            emit_rep()
        for chain in range(4):
            emit_pv_chain(state["prev"], chain)

    nc.compile()
    return nc


def _get_nc(reps=1, **kw):
    key = ("nc", reps, tuple(sorted(kw.items())))
    if key not in _CACHE:
        _CACHE[key] = _build_nc(reps, **kw)
    return _CACHE[key]


def _prep_core_inputs(q, k, v, m, core):
    b = core // (H // HPC)
    h0 = (core % (H // HPC)) * HPC
    qs = q[b, h0:h0 + HPC].transpose(0, 2, 1).astype(_BF16)   # [8, DK, S]
    ks = k[b, h0:h0 + HPC].transpose(0, 2, 1).astype(_BF16)
    q2 = np.empty((NPAIR, 2 * DK, S), dtype=_BF16)
    k2 = np.empty((NPAIR, 2 * DK, S), dtype=_BF16)
    for hp in range(NPAIR):
        q2[hp, :DK] = qs[2 * hp]
        q2[hp, DK:] = qs[2 * hp + 1]
        k2[hp, :DK] = ks[2 * hp]
        k2[hp, DK:] = ks[2 * hp + 1]
    vex = np.ones((S, HPC, VE), dtype=_BF16)
    vex[:, :, :DK] = v[b, h0:h0 + HPC].transpose(1, 0, 2)
    mT = m[b, 0].T
    maskT = mT[:POOL0 * KT].astype(_BF16)
    madd = ((~mT[(NKT - ID0) * KT:]).astype(np.float32) * -10000.0).astype(_BF16)
    exT = np.where(mT, np.float32(16256.0), np.float32(-29952.0)).astype(_BF16)
    ident = np.eye(KT, dtype=_BF16)
    return {
        "q2": q2, "k2": k2, "vex": vex,
        "maskT": np.ascontiguousarray(maskT),
        "maddT": np.ascontiguousarray(madd),
        "exT": np.ascontiguousarray(exT),
        "ident": ident,
    }


def kernel(query, key, value, mask):
    from concourse.bass_utils import run_bass_kernel_spmd

    q = np.asarray(query, dtype=np.float32)
    k = np.asarray(key, dtype=np.float32)
    v = np.asarray(value, dtype=np.float32)
    m = np.asarray(mask)

    nc = _get_nc()
    in_maps = [_prep_core_inputs(q, k, v, m, c) for c in range(NCORES)]
    res = run_bass_kernel_spmd(nc, in_maps, list(range(NCORES))).results

    out = np.empty((B, H, S, DK), dtype=np.float32)
    for c in range(NCORES):
        b = c // (H // HPC)
        h0 = (c % (H // HPC)) * HPC
        out[b, h0:h0 + HPC] = res[c]["out"]
    return out
